# revision 1
# baseline (speedup 1.0000x reference)
"""Trainium2 Bass kernel for nn_CliffordSirenLayer.

Computes, for full inputs (B=4, N=8192, M=512, IN=OUT=32):
    wT  = einsum('oid,cdk->oick', nan_to_num(weight), CLIFFORD_T)
    pre = einsum('bnic,oick->bnok', x, wT) + bias
    h   = softplus(q @ fw1.T + fb1); ls = clip(h @ fw2.T + fb2, 0, 5)
    dmin = min_m |q - atoms_m| (clamped); omega = 30*(1 + ls*exp(-dmin))
    out = sin(omega * pre)

Sharding: 8 cores; core c handles batch b=c//2, point half c%2 (4096 points).
All parameters are tiny and replicated; everything is embarrassingly parallel.

Device strategy per core (4096 pts = 32 chunks of 128 partitions):
  - Clifford linear as a dense [256,256] matmul, folded on host. Run as a
    bf16 3-pass split (x_hi@W_hi + x_hi@W_lo + x_lo@W_hi) which is f32-
    accurate, at 1 cycle/row on the PE (fp32 matmul is 4 cycles/row).
  - Distances via the |a|^2 - 2q.a + |q|^2 expansion: one K=4 matmul per
    chunk against precomputed atom features [-2a; |a|^2], 4-way row-tiled
    (tile_position) so 4 chunks share the PE array. DVE min-reduce.
  - freq-net h = q@fw1.T+fb1 as K=4 matmuls into one shared PSUM bank;
    softplus = Ln(Exp(h)+1) on ACT (no softplus table on this target).
  - sin needs manual range reduction (ACT Sin diverges past ~pi):
    t = pre*omega/(2pi) (ACT Copy w/ per-partition scale), k = int32(t)
    (round-to-nearest on DVE), r = t-k, out = ACT Sin(2pi*r).
"""

import sys

for _p in ("/opt/trn_rl_repo", "/root/.axon_site/_ro/trn_rl_repo"):
    if _p not in sys.path:
        sys.path.append(_p)

import numpy as np
import ml_dtypes

import concourse.bass as bass
import concourse.tile as tile
from concourse import bacc, mybir
from concourse.bass_utils import run_bass_kernel_spmd

F32 = mybir.dt.float32
BF16 = mybir.dt.bfloat16
I32 = mybir.dt.int32
AF = mybir.ActivationFunctionType
ALU = mybir.AluOpType
AX = mybir.AxisListType

B, N, M, IN, OUT = 4, 8192, 512, 32, 32
NCORES = 8
NLOC = (B * N) // NCORES          # 4096 points per core
CH = 128                          # points per chunk (partition dim)
NCH = NLOC // CH                  # 32 chunks
D = IN * 8                        # 256 contraction dim
DO = OUT * 8                      # 256 output dim

TWO_PI = 6.283185307179586
INV_2PI = 0.15915494309189535
SIREN_OMEGA_0 = 30.0


def _clifford_table():
    masks = [0, 1, 2, 4, 3, 5, 6, 7]
    idx = {m: i for i, m in enumerate(masks)}
    T = np.zeros((8, 8, 8), np.float64)
    for i, a in enumerate(masks):
        for j, b in enumerate(masks):
            s, aa = 1, a >> 1
            while aa:
                if bin(aa & b).count("1") & 1:
                    s = -s
                aa >>= 1
            T[i, j, idx[a ^ b]] = s
    return T


def _split_bf16(a32):
    hi = a32.astype(ml_dtypes.bfloat16)
    lo = (a32 - hi.astype(np.float32)).astype(ml_dtypes.bfloat16)
    return np.ascontiguousarray(hi), np.ascontiguousarray(lo)


def build_program(with_bias: bool, reps: int = 1):
    """Build + compile the per-core SPMD bass program."""
    nc = bacc.Bacc("TRN2", target_bir_lowering=False, debug=False, num_devices=1)

    XH = nc.dram_tensor("xh", [D, NLOC], BF16, kind="ExternalInput").ap()
    XL = nc.dram_tensor("xl", [D, NLOC], BF16, kind="ExternalInput").ap()
    WH = nc.dram_tensor("wh", [D, DO], BF16, kind="ExternalInput").ap()
    WL = nc.dram_tensor("wl", [D, DO], BF16, kind="ExternalInput").ap()
    QT4 = nc.dram_tensor("qt4", [128, NCH * 32], F32, kind="ExternalInput").ap()
    QTU = nc.dram_tensor("qtu", [4, NLOC], F32, kind="ExternalInput").ap()
    AF4 = nc.dram_tensor("af4", [128, M], F32, kind="ExternalInput").ap()
    FW1F4 = nc.dram_tensor("fw1f4", [128, 16], F32, kind="ExternalInput").ap()
    FW2R = nc.dram_tensor("fw2rep", [128, NCH * 16], F32, kind="ExternalInput").ap()
    Q2T = nc.dram_tensor("q2t", [128, NCH], F32, kind="ExternalInput").ap()
    FB2S = nc.dram_tensor("fb2s", [128, 1], F32, kind="ExternalInput").ap()
    if with_bias:
        BH = nc.dram_tensor("bh", [1, DO], BF16, kind="ExternalInput").ap()
        BL = nc.dram_tensor("bl", [1, DO], BF16, kind="ExternalInput").ap()
    Y = nc.dram_tensor("y", [NLOC, DO], F32, kind="ExternalOutput").ap()

    with tile.TileContext(nc) as tc:
        for _rep in range(reps):
            _emit_body(nc, tc, XH, XL, QT4, QTU, AF4, FW1F4, FW2R, Q2T, FB2S,
                       WH, WL, BH if with_bias else None,
                       BL if with_bias else None, Y)

    nc.compile()
    return nc


def _emit_body(nc, tc, XH, XL, QT4, QTU, AF4, FW1F4, FW2R, Q2T, FB2S, WH, WL, BH, BL, Y):
    with (
        tc.tile_pool(name="const", bufs=1) as cp,
        tc.tile_pool(name="xin", bufs=1) as xp,
        tc.tile_pool(name="work", bufs=2) as wp,
        tc.tile_pool(name="rr", bufs=3) as rp,
        tc.tile_pool(name="outp", bufs=2) as op,
        tc.tile_pool(name="psA", bufs=1, space="PSUM") as psA,
        tc.tile_pool(name="psB", bufs=3, space="PSUM") as psB,
    ):
        # ---- constant/parameter loads ----
        qt4 = cp.tile([128, NCH * 32], F32, tag="qt4")
        nc.sync.dma_start(qt4[:], QT4[:])
        qtu = cp.tile([4, NLOC], F32, tag="qtu")
        nc.sync.dma_start(qtu[:], QTU[:])
        af4 = cp.tile([128, M], F32, tag="af4")
        nc.sync.dma_start(af4[:], AF4[:])
        fw1f4 = cp.tile([128, 16], F32, tag="fw1f4")
        nc.sync.dma_start(fw1f4[:], FW1F4[:])
        fw2r = cp.tile([128, NCH * 16], F32, tag="fw2r")
        nc.sync.dma_start(fw2r[:], FW2R[:])
        q2t = cp.tile([128, NCH], F32, tag="q2t")
        nc.sync.dma_start(q2t[:], Q2T[:])
        fb2s = cp.tile([128, 1], F32, tag="fb2s")
        nc.sync.dma_start(fb2s[:], FB2S[:])

        wh = [cp.tile([128, DO], BF16, tag=f"wh{k}", name=f"wh{k}") for k in range(2)]
        wl = [cp.tile([128, DO], BF16, tag=f"wl{k}", name=f"wl{k}") for k in range(2)]
        for k in range(2):
            nc.sync.dma_start(wh[k][:], WH[128 * k:128 * (k + 1), :])
            nc.sync.dma_start(wl[k][:], WL[128 * k:128 * (k + 1), :])

        xh = [xp.tile([128, NLOC], BF16, tag=f"xh{k}", name=f"xh{k}") for k in range(2)]
        xl = [xp.tile([128, NLOC], BF16, tag=f"xl{k}", name=f"xl{k}") for k in range(2)]
        for k in range(2):
            nc.sync.dma_start(xh[k][:], XH[128 * k:128 * (k + 1), :])
            nc.sync.dma_start(xl[k][:], XL[128 * k:128 * (k + 1), :])

        if BH is not None:
            ones = cp.tile([1, CH], BF16, tag="ones")
            nc.vector.memset(ones[:], 1.0)
            bh = cp.tile([1, DO], BF16, tag="bh")
            nc.sync.dma_start(bh[:], BH[:])
            bl = cp.tile([1, DO], BF16, tag="bl")
            nc.sync.dma_start(bl[:], BL[:])

        # ---- phase A: omega ----
        dmin = cp.tile([128, NCH], F32, tag="dmin")
        h_ps = psA.tile([128, NCH * 16], F32, tag="hps")   # one bank, 32x16 cols

        for t in range(NCH):
            g, tg = t % 4, t // 4
            lhs = qt4[32 * g:32 * g + 4, bass.ts(tg, 128)]
            d2 = psA.tile([128, M], F32, tag=f"d2_{g}")
            nc.tensor.matmul(d2[:], lhs, af4[32 * g:32 * g + 4, :],
                             start=True, stop=True, tile_position=(32 * g, 0))
            nc.vector.tensor_reduce(dmin[:, t:t + 1], d2[:], axis=AX.X, op=ALU.min)
            # h matmul stays at row-group 0 (unpacked qtu): concurrent
            # row-group matmuls into one shared PSUM bank crash the HW.
            nc.tensor.matmul(h_ps[:, 16 * t:16 * (t + 1)],
                             qtu[:, bass.ts(t, CH)], fw1f4[0:4, :],
                             start=True, stop=True)

        d2c = cp.tile([128, NCH], F32, tag="d2c")
        nc.vector.tensor_add(d2c[:], dmin[:], q2t[:])
        nc.vector.tensor_scalar_max(d2c[:], d2c[:], 1e-4)
        dist = cp.tile([128, NCH], F32, tag="dist")
        nc.scalar.activation(dist[:], d2c[:], AF.Sqrt)
        e = cp.tile([128, NCH], F32, tag="e")
        nc.scalar.activation(e[:], dist[:], AF.Exp, scale=-1.0)

        he = wp.tile([128, NCH * 16], F32, tag="he")
        nc.scalar.activation(he[:], h_ps[:], AF.Exp)
        hsp = wp.tile([128, NCH * 16], F32, tag="hsp")
        nc.scalar.activation(hsp[:], he[:], AF.Ln, bias=1.0)

        prod = wp.tile([128, NCH * 16], F32, tag="prod")
        nc.vector.tensor_mul(prod[:], hsp[:], fw2r[:])
        lsr = cp.tile([128, NCH], F32, tag="lsr")
        nc.vector.tensor_reduce(lsr[:], prod[:].rearrange("p (t j) -> p t j", j=16),
                                axis=AX.X, op=ALU.add)
        ls = cp.tile([128, NCH], F32, tag="ls")
        # ls = min(max(lsr + fb2, 0), 5)
        nc.vector.tensor_scalar(ls[:], lsr[:], fb2s[:], 0.0, ALU.add, ALU.max)
        nc.vector.tensor_scalar_min(ls[:], ls[:], 5.0)
        om2p = cp.tile([128, NCH], F32, tag="om2p")   # omega / (2*pi)
        nc.vector.tensor_mul(om2p[:], ls[:], e[:])
        nc.vector.tensor_scalar(om2p[:], om2p[:], SIREN_OMEGA_0 * INV_2PI,
                                SIREN_OMEGA_0 * INV_2PI, ALU.mult, ALU.add)

        # ---- phase B: clifford matmul + modulated sin ----
        for j in range(NCH // 8):
            osb = op.tile([128, 8, DO], F32, tag="osb")
            for tt in range(8):
                t = 8 * j + tt
                pre = psB.tile([128, DO], F32, tag="pre")
                nc.tensor.matmul(pre[:], xh[0][:, bass.ts(t, CH)], wh[0][:], start=True, stop=False)
                nc.tensor.matmul(pre[:], xh[1][:, bass.ts(t, CH)], wh[1][:], start=False, stop=False)
                nc.tensor.matmul(pre[:], xh[0][:, bass.ts(t, CH)], wl[0][:], start=False, stop=False)
                nc.tensor.matmul(pre[:], xh[1][:, bass.ts(t, CH)], wl[1][:], start=False, stop=False)
                nc.tensor.matmul(pre[:], xl[0][:, bass.ts(t, CH)], wh[0][:], start=False, stop=False)
                last = BH is None
                nc.tensor.matmul(pre[:], xl[1][:, bass.ts(t, CH)], wh[1][:], start=False, stop=last)
                if BH is not None:
                    nc.tensor.matmul(pre[:], ones[:], bh[:], start=False, stop=False)
                    nc.tensor.matmul(pre[:], ones[:], bl[:], start=False, stop=True)

                tsb = rp.tile([128, DO], F32, tag="tsb")
                nc.scalar.activation(tsb[:], pre[:], AF.Copy, scale=om2p[:, t:t + 1])
                ki = rp.tile([128, DO], I32, tag="ki")
                nc.vector.tensor_copy(ki[:], tsb[:])
                r = rp.tile([128, DO], F32, tag="r")
                nc.vector.tensor_sub(r[:], tsb[:], ki[:])
                nc.scalar.activation(osb[:, tt, :], r[:], AF.Sin, scale=TWO_PI)

            dst = Y[1024 * j:1024 * (j + 1), :].rearrange("(c p) o -> p c o", p=128)
            nc.sync.dma_start(dst, osb[:])


def prepare_inputs(x, query_coords, atomic_coords, weight, bias, fw1, fb1, fw2, fb2):
    """Host-side prep: fold the Clifford table into W, split bf16, shard."""
    T = _clifford_table()
    w64 = np.nan_to_num(np.asarray(weight)).astype(np.float64)
    Wm = np.einsum("oid,cdk->icok", w64, T).reshape(D, DO).astype(np.float32)
    wh, wl = _split_bf16(Wm)

    bias_flat = np.asarray(bias).astype(np.float32).reshape(DO)
    with_bias = bool(np.any(bias_flat))
    bh, bl = _split_bf16(bias_flat.reshape(1, DO))

    fw1 = np.asarray(fw1).astype(np.float64)
    fb1 = np.asarray(fb1).astype(np.float64)
    fw2 = np.asarray(fw2).astype(np.float64).reshape(16)
    fb2 = float(np.asarray(fb2).reshape(()))

    fw1_feat = np.concatenate([fw1.T, fb1.reshape(1, 16)], axis=0)  # [4,16]
    fw1f4 = np.zeros((128, 16), np.float32)
    for g in range(4):
        fw1f4[32 * g:32 * g + 4, :] = fw1_feat
    fw2rep = np.tile(fw2.astype(np.float32), (128, NCH))            # [128, 512]
    fb2s = np.full((128, 1), fb2, np.float32)

    x = np.asarray(x)
    q_all = np.asarray(query_coords).astype(np.float64)
    a_all = np.asarray(atomic_coords).astype(np.float64)

    in_maps = []
    for c in range(NCORES):
        b, half = c // 2, c % 2
        sl = slice(half * NLOC, (half + 1) * NLOC)
        xc = np.ascontiguousarray(x[b, sl].reshape(NLOC, D).T.astype(np.float32))
        xh, xl = _split_bf16(xc)

        q = q_all[b, sl]                                            # [4096, 3]
        qT_aug = np.concatenate([q.T, np.ones((1, NLOC))], axis=0)  # [4, 4096]
        qa = qT_aug.reshape(4, NCH // 4, 4, CH)                     # [k, tg, g, j]
        qt4 = np.zeros((128, NCH * 32), np.float32)
        for g in range(4):
            for k in range(4):
                qt4[32 * g + k, :] = qa[k, :, g, :].reshape(-1)
        qtu = qT_aug.astype(np.float32)

        a = a_all[b]                                                # [512, 3]
        feat = np.concatenate([-2.0 * a.T, (a * a).sum(1).reshape(1, M)], axis=0)
        af4 = np.zeros((128, M), np.float32)
        for g in range(4):
            af4[32 * g:32 * g + 4, :] = feat

        q2 = (q * q).sum(1).astype(np.float32)                      # [4096]
        q2t = np.ascontiguousarray(q2.reshape(NCH, CH).T)           # [128, 32]

        m = {
            "xh": xh, "xl": xl, "wh": wh, "wl": wl,
            "qt4": qt4, "qtu": qtu, "af4": af4, "fw1f4": fw1f4, "fw2rep": fw2rep,
            "q2t": q2t, "fb2s": fb2s,
        }
        if with_bias:
            m["bh"] = bh
            m["bl"] = bl
        in_maps.append(m)
    return in_maps, with_bias


_PROGRAM_CACHE = {}


def get_program(with_bias: bool, reps: int = 1):
    key = (with_bias, reps)
    if key not in _PROGRAM_CACHE:
        _PROGRAM_CACHE[key] = build_program(with_bias, reps)
    return _PROGRAM_CACHE[key]


def assemble_output(results):
    out = np.empty((B, N, OUT, 8), np.float32)
    for c in range(NCORES):
        b, half = c // 2, c % 2
        out[b, half * NLOC:(half + 1) * NLOC] = results[c]["y"].reshape(NLOC, OUT, 8)
    return out


def kernel(x, query_coords, atomic_coords, weight, bias, fw1, fb1, fw2, fb2):
    in_maps, with_bias = prepare_inputs(
        x, query_coords, atomic_coords, weight, bias, fw1, fb1, fw2, fb2)
    nc = get_program(with_bias)
    res = run_bass_kernel_spmd(nc, in_maps, core_ids=list(range(NCORES)))
    return assemble_output(res.results)


if __name__ == "__main__":
    # tiny self-check against a numpy model
    rng = np.random.default_rng(0)
    print("kernel module loaded; run test.py for the full check")

